# revision 4
# baseline (speedup 1.0000x reference)
"""DoSRUconv Trainium2 kernel: 3x3x3 conv -> 6 gates -> bidirectional SRU scan.

Sharding: H (dim 3) split across 8 cores, 16 rows each; 3x3x3 conv halo is
resolved host-side by handing each core a zero-padded 18-row slab, so cores
are fully independent (no collectives).

Per-core pipeline:
  - conv as matmuls over contraction (ci, dt, dh, dw) = 432(+bias row):
    3 K=128 blocks (8 (dh,dw)-taps x 16ci shifted replicas of x, dt via
    t-slice offset) + one K=49 leftover block ((1,1) tap x 3 dt + ones row
    carrying the bias).  float32r => 1 cycle/row at N=256.
  - replica tiles hold [2h x 130w] contiguous row-pairs per t-slot at pitch
    264; the per-tap (dh, dw) shift is baked in via the DMA's src row range
    and dst offset (1-dw), so one DMA per tap group loads both h rows.
  - PE-transpose of gate tiles [96, 128] -> [128(w), 96] per (t, h-row).
  - gates stored pixel-major [128w, plane, c, 32 t-slots]; tanh/sigmoid
    applied there with full 128-lane ACT ops.
  - SRU recurrence via tensor_tensor_scan (state = f*state - b with
    b=(f-1)*wx); 16 channel segments chained in one [128, 512] scan with
    zero-f/zero-b separator columns; backward direction stored t-reversed,
    reading wx through negative-stride views.
  - output written to DRAM as [B, H, W, C, T] (one DMA per (b, chunk, h));
    host transposes back to [B, C, T, H, W].
"""

import os

import numpy as np

import concourse.bass as bass
import concourse.mybir as mybir
import concourse.tile as tile
from concourse import bacc
from concourse.bass_utils import run_bass_kernel_spmd
from concourse.masks import make_identity

F32 = mybir.dt.float32
F32R = mybir.dt.float32r
ALU = mybir.AluOpType
ACTF = mybir.ActivationFunctionType

B, CIN, COUT, T, H, W = 2, 16, 16, 31, 128, 128
NCORES = 8
HSLAB = H // NCORES                  # 16
HC = 2                               # h-rows per conv chunk (matmul N = HC*128)
NCHUNK = HSLAB // HC                 # 8
TP, WP = T + 2, W + 2                # padded dims: 33 t-slices, 130 w
SLOTS = 32                           # t-slots per channel segment (31 t + pad)
NPIX = HC * W                        # 256
RPITCH = 264                         # replica slot pitch: 2*130 + 2 slack
LFULL = bool(int(os.environ.get("K_LFULL", "1")))

# 8 main taps (dh, dw), lexicographic minus (1,1); leftover tap is (1,1)
MAIN_TAPS = [(dh, dw) for dh in (-1, 0, 1) for dw in (-1, 0, 1)][:8]
# gate order [Wx, X, ft, rt, ft2, rt2] from reference split order
# [Wx, ft, ft2, rt, rt2, X]
GPERM = [0, 5, 1, 3, 2, 4]
# Gp plane indices (pixel-major gate store); tanh planes 0:2, sigmoid 2:6
PL_WX, PL_X, PL_FT, PL_RT, PL_FT2, PL_RT2 = range(6)
THALVES = [(0, 16), (16, T)]


def _flat2(ap):
    return ap.rearrange("p a b -> p (a b)")


def _rev_last(ap, count=31, start=30):
    """View with the last (unit-stride) dim replaced by a reversed run of
    `count` starting at index `start` (descending)."""
    new_ap = [list(d) for d in ap.ap]
    assert new_ap[-1][0] == 1
    new_ap[-1] = [-1, count]
    return bass.AP(tensor=ap.tensor, offset=ap.offset + start, ap=new_ap)


def _rhs_view(rep, slot):
    """Matmul rhs [P, HC, W] from a replica tile [P, slots, RPITCH]:
    element (r, w) at slot*RPITCH + 2 + 130*r + w."""
    base = rep[:, slot]          # [P, RPITCH]
    ap = [list(base.ap[0]), [130, HC], [1, W]]
    return bass.AP(tensor=base.tensor, offset=base.offset + 2, ap=ap)


def build_nc(reps=1):
    nc = bacc.Bacc("TRN2", target_bir_lowering=False, debug=False)

    xs = nc.dram_tensor("xs", [B, CIN, TP, HSLAB + 2, WP], F32R,
                        kind="ExternalInput").ap()
    wmain_d = nc.dram_tensor("wmain", [128, 3, 96], F32R,
                             kind="ExternalInput").ap()
    wleft_d = nc.dram_tensor("wleft", [49, 96], F32R,
                             kind="ExternalInput").ap()
    ones_d = nc.dram_tensor("ones", [1, T * RPITCH], F32R,
                            kind="ExternalInput").ap()
    # stored [b, h, w, c, t]; host transposes back to [b, c, t, h, w]
    out_d = nc.dram_tensor("out", [B, HSLAB, W, COUT, T], F32,
                           kind="ExternalOutput").ap()

    with tile.TileContext(nc) as tc:
        with (
            tc.tile_pool(name="const", bufs=1) as constp,
            tc.tile_pool(name="mrep", bufs=int(os.environ.get("K_MREP_BUFS", "2"))) as mpool,
            tc.tile_pool(name="lrep", bufs=2) as lpool,
            tc.tile_pool(name="gc", bufs=int(os.environ.get("K_GC_BUFS", "2"))) as gcpool,
            tc.tile_pool(name="gp", bufs=int(os.environ.get("K_GP_BUFS", "1"))) as gppool,
            tc.tile_pool(name="scr", bufs=int(os.environ.get("K_SCR_BUFS", "2"))) as scrp,
            tc.tile_pool(name="ot", bufs=int(os.environ.get("K_OT_BUFS", "2"))) as otpool,
            tc.tile_pool(name="mmps", bufs=int(os.environ.get("K_MM_BUFS", "2")), space="PSUM") as mmpool,
            tc.tile_pool(name="trps", bufs=int(os.environ.get("K_TR_BUFS", "2")), space="PSUM") as trpool,
        ):
            wmain_sb = constp.tile([128, 3, 96], F32R)
            nc.sync.dma_start(out=wmain_sb, in_=wmain_d)
            wleft_sb = constp.tile([49, 96], F32R)
            nc.sync.dma_start(out=wleft_sb, in_=wleft_d)
            ident = constp.tile([128, 128], F32)
            make_identity(nc, ident)
            zscr6 = constp.tile([128, 6, 16], F32)
            nc.vector.memset(zscr6, 0.0)
            zscr16 = constp.tile([128, 16], F32)
            nc.vector.memset(zscr16, 0.0)

            _lim = int(os.environ.get("K_CHUNK_LIMIT", "0"))
            _pairs = [(rep, b, c) for rep in range(reps)
                      for b in range(B) for c in range(NCHUNK)]
            if _lim:
                _pairs = _pairs[:_lim]
            for rep, b, chunk in _pairs:
                    h0 = chunk * HC
                    gp = [gppool.tile([128, 6, 16, SLOTS], F32, tag=f"gp{r}",
                                      name=f"gp{r}_{rep}_{b}_{chunk}")
                          for r in range(HC)]
                    for r in range(HC):
                        nc.gpsimd.tensor_copy(gp[r][:, :, :, 31:32],
                                              zscr6.unsqueeze(3))
                    ot = otpool.tile([128, HC, COUT, T], F32, tag="ot")

                    for (ta, tb) in THALVES:
                        nsl = tb - ta + 2
                        # one DMA per tap group: [nsl, 2h, 130w] contiguous
                        # rows; dst offset 1-dw bakes in the w-shift, src
                        # row range bakes in the h-shift.
                        mrep = mpool.tile([128, 18, RPITCH], F32R, tag="mrep")
                        _split = int(os.environ.get("K_DMA_SPLIT", "1"))
                        for g, (dh, dw) in enumerate(MAIN_TAPS):
                            dma_eng = nc.scalar if (_split and g % 2) else nc.sync
                            dma_eng.dma_start(
                                out=mrep[g * 16:(g + 1) * 16, :nsl,
                                         1 - dw:1 - dw + 2 * WP],
                                in_=xs[b, :, ta:ta + nsl,
                                       h0 + 1 + dh:h0 + 3 + dh, :].rearrange(
                                    "c t h w -> c t (h w)"))
                        if LFULL:
                            if ta == 0:
                                lrep = lpool.tile([49, T, RPITCH], F32R,
                                                  tag="lrep",
                                                  name=f"lrepf{rep}_{b}_{chunk}")
                                for dtg in range(3):
                                    nc.sync.dma_start(
                                        out=lrep[dtg * 16:(dtg + 1) * 16,
                                                 :, 0:2 * WP],
                                        in_=xs[b, :, dtg:T + dtg,
                                               h0 + 2:h0 + 4, :].rearrange(
                                            "c t h w -> c t (h w)"))
                                nc.sync.dma_start(
                                    out=_flat2(lrep[48:49, :, :]),
                                    in_=ones_d[:, :T * RPITCH])
                        else:
                            lrep = lpool.tile([49, 16, RPITCH], F32R,
                                              tag="lrep")
                            for dtg in range(3):
                                nc.sync.dma_start(
                                    out=lrep[dtg * 16:(dtg + 1) * 16,
                                             :tb - ta, 0:2 * WP],
                                    in_=xs[b, :, ta + dtg:tb + dtg,
                                           h0 + 2:h0 + 4, :].rearrange(
                                        "c t h w -> c t (h w)"))
                            nc.sync.dma_start(
                                out=_flat2(lrep[48:49, :16, :]),
                                in_=ones_d[:, :16 * RPITCH])

                        gc = gcpool.tile([96, 16, NPIX], F32, tag="gc")

                        # conv matmuls + raw eviction, 4-t psum groups
                        for gi, tg in enumerate(range(ta, tb, 4)):
                            gs = min(4, tb - tg)
                            ps = mmpool.tile([96, 4, NPIX], F32, tag="mmps")
                            for tt in range(tg, tg + gs):
                                s = tt - ta
                                pslot = ps[:, tt - tg, :]
                                for dt in range(3):
                                    nc.tensor.matmul(
                                        pslot,
                                        wmain_sb[:, dt, :],
                                        _rhs_view(mrep, s + dt),
                                        start=(dt == 0), stop=False)
                                nc.tensor.matmul(
                                    pslot,
                                    wleft_sb,
                                    _rhs_view(lrep, tt if LFULL else s),
                                    start=False, stop=True)
                            dst = gc[:, tg - ta:tg - ta + gs, :]
                            src = ps[:, :gs, :]
                            _ev = int(os.environ.get("K_EVICT", "2"))
                            if (_ev == 1 or (_ev == 0 and gi % 2 == 0)
                                    or (_ev == 3 and gi % 3 == 0)):
                                nc.scalar.activation(dst, src, ACTF.Copy)
                            else:
                                nc.vector.tensor_copy(dst, src)

                        # transpose to pixel-major + plane scatter
                        _csz = int(os.environ.get("K_CS", "8"))
                        for cg in range(ta, tb, _csz):
                            cs = min(_csz, tb - cg)
                            for r in range(HC):
                                trp = trpool.tile([128, _csz, 128], F32,
                                                  tag="trps")
                                for j in range(cs):
                                    nc.tensor.transpose(
                                        trp[:, j, 0:96],
                                        gc[:, cg - ta + j,
                                           r * W:(r + 1) * W],
                                        ident[:96, :96])
                                tv = trp[:, :cs, 0:96]
                                gpr = gp[r]
                                # fwd planes wx,X,ft,rt (cols 0:64), slot=t
                                nc.vector.tensor_copy(
                                    gpr[:, 0:4, :, cg:cg + cs],
                                    tv[:, :, 0:64].rearrange(
                                        "p t (g c) -> p g c t", g=4))
                                # bwd planes ft2,rt2 (cols 64:96), slot=30-t
                                nc.vector.tensor_copy(
                                    _rev_last(gpr[:, PL_FT2:PL_RT2 + 1],
                                              count=cs, start=30 - cg),
                                    tv[:, :, 64:96].rearrange(
                                        "p t (g c) -> p g c t", g=2))

                    # activations per t-half slot range (overlaps the
                    # other half's conv); fwd slots [ta,tb), bwd [31-tb,31-ta)
                    if int(os.environ.get("K_ACT_SPLIT", "0")):
                        for (ta, tb) in THALVES:
                            for r in range(HC):
                                gpr = gp[r]
                                for (p0, p1, fn) in ((0, 2, ACTF.Tanh),
                                                     (2, 6, ACTF.Sigmoid)):
                                    fv = gpr[:, p0:p1, :, ta:tb].rearrange(
                                        "p a c s -> p (a c) s")
                                    nc.scalar.activation(fv, fv, fn)
                                    bv = gpr[:, p0:p1, :,
                                             31 - tb:31 - ta].rearrange(
                                        "p a c s -> p (a c) s")
                                    nc.scalar.activation(bv, bv, fn)
                    # scan phase for this (b, chunk)
                    for r in range(HC):
                        gpr = gp[r]
                        if not int(os.environ.get("K_ACT_SPLIT", "0")):
                            nc.scalar.activation(
                                _flat2(gpr[:, 0:2].rearrange(
                                    "p a c s -> p (a c) s")),
                                _flat2(gpr[:, 0:2].rearrange(
                                    "p a c s -> p (a c) s")),
                                ACTF.Tanh)
                            nc.scalar.activation(
                                _flat2(gpr[:, 2:6].rearrange(
                                    "p a c s -> p (a c) s")),
                                _flat2(gpr[:, 2:6].rearrange(
                                    "p a c s -> p (a c) s")),
                                ACTF.Sigmoid)
                        nc.gpsimd.tensor_copy(gpr[:, PL_FT, :, 31:32],
                                              zscr16.unsqueeze(2))
                        nc.gpsimd.tensor_copy(gpr[:, PL_FT2, :, 31:32],
                                              zscr16.unsqueeze(2))

                        cf = scrp.tile([128, 16, SLOTS], F32, tag="cf")
                        cb = scrp.tile([128, 16, SLOTS], F32, tag="cb")
                        bb_ = scrp.tile([128, 16, SLOTS], F32, tag="bb")
                        wx_pl = gpr[:, PL_WX]
                        for (pf, wx_in, cdst) in (
                                (PL_FT, wx_pl[:, :, 1:31], cf),
                                (PL_FT2, _rev_last(wx_pl, 30, 29), cb)):
                            f_pl = gpr[:, pf]
                            nc.vector.tensor_scalar_sub(
                                bb_[:, :, 0:1], f_pl[:, :, 0:1], 1.0)
                            nc.vector.scalar_tensor_tensor(
                                out=bb_[:, :, 1:31], in0=f_pl[:, :, 1:31],
                                scalar=1.0, in1=wx_in,
                                op0=ALU.subtract, op1=ALU.mult)
                            nc.gpsimd.tensor_copy(bb_[:, :, 31:32],
                                                  zscr16.unsqueeze(2))
                            nc.vector.tensor_tensor_scan(
                                out=_flat2(cdst), data0=_flat2(f_pl),
                                data1=_flat2(bb_), initial=0.0,
                                op0=ALU.mult, op1=ALU.subtract)

                        s1 = scrp.tile([128, 16, 31], F32, tag="s1")
                        s2 = scrp.tile([128, 16, SLOTS], F32, tag="s2")
                        dd = scrp.tile([128, 16, 31], F32, tag="dd")
                        ee = scrp.tile([128, 16, 31], F32, tag="ee")
                        nc.gpsimd.tensor_mul(
                            s1, gpr[:, PL_RT, :, 0:31], cf[:, :, 0:31])
                        nc.gpsimd.tensor_mul(
                            s2[:, :, 0:31], gpr[:, PL_RT2, :, 0:31],
                            cb[:, :, 0:31])
                        nc.gpsimd.tensor_add(
                            dd, gpr[:, PL_RT, :, 0:31],
                            _rev_last(gpr[:, PL_RT2], 31, 30))
                        nc.vector.scalar_tensor_tensor(
                            out=ee, in0=dd, scalar=2.0,
                            in1=gpr[:, PL_X, :, 0:31],
                            op0=ALU.subtract, op1=ALU.mult)
                        nc.gpsimd.tensor_add(s1, s1, _rev_last(s2, 31, 30))
                        nc.gpsimd.tensor_sub(ot[:, r], s1, ee)

                    # output: [128w, c, t] per h-row -> contiguous DRAM run
                    for r in range(HC):
                        nc.scalar.dma_start(
                            out=out_d[b, h0 + r, :, :, :],
                            in_=ot[:, r])
    nc.compile()
    return nc


_NC_CACHE = {}


def _get_nc(reps=1):
    if reps not in _NC_CACHE:
        _NC_CACHE[reps] = build_nc(reps)
    return _NC_CACHE[reps]


def make_host_inputs(x, conv_w, conv_b):
    """Pad x, permute/flatten weights. Returns (xp, wmain, wleft, ones)."""
    x = np.asarray(x, np.float32)
    conv_w = np.asarray(conv_w, np.float32)
    conv_b = np.asarray(conv_b, np.float32)

    xp = np.zeros((B, CIN, TP, H + 2, WP), np.float32)
    xp[:, :, 1:1 + T, 1:1 + H, 1:1 + W] = x

    wp = conv_w.reshape(6, COUT, CIN, 3, 3, 3)[GPERM].reshape(
        96, CIN, 3, 3, 3)
    bp = conv_b.reshape(6, COUT)[GPERM].reshape(96)

    wmain = np.zeros((128, 3, 96), np.float32)
    for g, (dh, dw) in enumerate(MAIN_TAPS):
        for dt in range(3):
            wmain[g * 16:(g + 1) * 16, dt, :] = wp[:, :, dt, dh + 1, dw + 1].T
    wleft = np.zeros((49, 96), np.float32)
    for dtg in range(3):
        wleft[dtg * 16:(dtg + 1) * 16, :] = wp[:, :, dtg, 2, 2].T
    wleft[48, :] = bp
    ones = np.ones((1, T * RPITCH), np.float32)
    return xp, wmain, wleft, ones


def core_inputs(xp, wmain, wleft, ones, k):
    return {
        "xs": np.ascontiguousarray(
            xp[:, :, :, k * HSLAB:k * HSLAB + HSLAB + 2, :]),
        "wmain": wmain,
        "wleft": wleft,
        "ones": ones,
    }


def kernel(x, conv_w, conv_b):
    nc = _get_nc()
    xp, wmain, wleft, ones = make_host_inputs(x, conv_w, conv_b)
    in_maps = [core_inputs(xp, wmain, wleft, ones, k) for k in range(NCORES)]
    res = run_bass_kernel_spmd(nc, in_maps, list(range(NCORES)))
    outs = [res.results[k]["out"].transpose(0, 3, 4, 1, 2)
            for k in range(NCORES)]
    return np.concatenate(outs, axis=3)



# revision 10
# speedup vs baseline: 4.2306x; 4.2306x over previous
"""DoSRUconv Trainium2 kernel: 3x3x3 conv -> 6 gates -> bidirectional SRU scan.

Sharding: H (dim 3) split across 8 cores, 16 rows each; 3x3x3 conv halo is
resolved host-side by handing each core a zero-padded 18-row slab, so cores
are fully independent (no collectives).

Per-core pipeline (v2, bf16 datapath):
  - x and conv weights cast to bf16 on host; conv as bf16 matmuls over
    contraction (ci, dt, dh, dw) = 432(+bias row): 3 K=128 blocks (8
    (dh,dw)-taps x 16ci shifted replicas of x, dt via t-slice offset) + one
    K=49 leftover block ((1,1) tap x 3 dt + ones row carrying the bias);
    f32 PSUM accumulate.  bf16 => 1 cycle/row on PE at any N.
  - replica tiles hold [2h x 130w] contiguous row-pairs per t-slot (full-T,
    33 slots) at pitch 264; the per-tap (dh, dw) shift is baked in via the
    DMA's src row range and dst offset (1-dw), so one DMA per tap loads the
    whole band.  bf16 halves all HBM traffic.
  - PSUM evicted to bf16 gate tiles (DVE/ACT split), PE-transposed
    [96, 128] -> [128(w), 96] per (t, h-row).
  - tanh/sigmoid fused into the post-transpose plane scatter on the ACT
    engine (3 ops per (h-row, t-group): tanh cols 0:32, sigmoid 32:64,
    sigmoid 64:96 written t-reversed for the backward direction), so no
    separate activation pass exists.
  - SRU recurrence via tensor_tensor_scan in f32 (state = f*state - b with
    b=(f-1)*wx); 16 channel segments chained in one [128, 512] scan with
    zero-f/zero-b separator columns; backward direction stored t-reversed,
    reading wx through negative-stride views.
  - output written to DRAM as [B, H, W, C, T] f32 (one DMA per (b, chunk,
    h)); host transposes back to [B, C, T, H, W].
"""

import os

import numpy as np

import concourse.bass as bass
import concourse.mybir as mybir
import concourse.tile as tile
from concourse import bacc
from concourse.bass_utils import run_bass_kernel_spmd
from concourse.masks import make_identity

F32 = mybir.dt.float32
BF16 = mybir.dt.bfloat16
ALU = mybir.AluOpType
ACTF = mybir.ActivationFunctionType

B, CIN, COUT, T, H, W = 2, 16, 16, 31, 128, 128
NCORES = 8
HSLAB = H // NCORES                  # 16
HC = 2                               # h-rows per conv chunk (matmul N = HC*128)
NCHUNK = HSLAB // HC                 # 8
TP, WP = T + 2, W + 2                # padded dims: 33 t-slices, 130 w
SLOTS = 32                           # t-slots per channel segment (31 t + pad)
NPIX = HC * W                        # 256
RPITCH = 264                         # replica slot pitch: 2*130 + 2 slack

# 8 main taps (dh, dw), lexicographic minus (1,1); leftover tap is (1,1)
MAIN_TAPS = [(dh, dw) for dh in (-1, 0, 1) for dw in (-1, 0, 1)][:8]
# gate order [Wx, X, ft, rt, ft2, rt2] from reference split order
# [Wx, ft, ft2, rt, rt2, X]
GPERM = [0, 5, 1, 3, 2, 4]
# Gp plane indices (pixel-major gate store); tanh planes 0:2, sigmoid 2:6
PL_WX, PL_X, PL_FT, PL_RT, PL_FT2, PL_RT2 = range(6)


def _flat2(ap):
    return ap.rearrange("p a b -> p (a b)")


def _rev_last(ap, count=31, start=30):
    """View with the last (unit-stride) dim replaced by a reversed run of
    `count` starting at index `start` (descending)."""
    new_ap = [list(d) for d in ap.ap]
    assert new_ap[-1][0] == 1
    new_ap[-1] = [-1, count]
    return bass.AP(tensor=ap.tensor, offset=ap.offset + start, ap=new_ap)


def _rhs_view(rep, slot):
    """Matmul rhs [P, HC, W] from a replica tile [P, slots, RPITCH]:
    element (r, w) at slot*RPITCH + 2 + 130*r + w."""
    base = rep[:, slot]          # [P, RPITCH]
    ap = [list(base.ap[0]), [130, HC], [1, W]]
    return bass.AP(tensor=base.tensor, offset=base.offset + 2, ap=ap)


def build_nc(reps=1):
    nc = bacc.Bacc("TRN2", target_bir_lowering=False, debug=False)

    xs = nc.dram_tensor("xs", [B, CIN, TP, HSLAB + 2, WP], BF16,
                        kind="ExternalInput").ap()
    wmain_d = nc.dram_tensor("wmain", [128, 3, 96], BF16,
                             kind="ExternalInput").ap()
    wleft_d = nc.dram_tensor("wleft", [49, 96], BF16,
                             kind="ExternalInput").ap()
    ones_d = nc.dram_tensor("ones", [1, T * RPITCH], BF16,
                            kind="ExternalInput").ap()
    # stored [b, h, w, c, t]; host transposes back to [b, c, t, h, w]
    out_d = nc.dram_tensor("out", [B, HSLAB, W, COUT, T], F32,
                           kind="ExternalOutput").ap()

    ev_mode = int(os.environ.get("K_EVICT", "0"))   # 0 alt, 1 ACT, 2 DVE
    csz = int(os.environ.get("K_CS", "16"))

    with tile.TileContext(nc) as tc:
        with (
            tc.tile_pool(name="const", bufs=1) as constp,
            tc.tile_pool(name="mrep", bufs=int(os.environ.get("K_MREP_BUFS", "2"))) as mpool,
            tc.tile_pool(name="lrep", bufs=1) as lpool,
            tc.tile_pool(name="gc", bufs=int(os.environ.get("K_GC_BUFS", "2"))) as gcpool,
            tc.tile_pool(name="gp", bufs=int(os.environ.get("K_GP_BUFS", "1"))) as gppool,
            tc.tile_pool(name="scr", bufs=int(os.environ.get("K_SCR_BUFS", "2"))) as scrp,
            tc.tile_pool(name="ot", bufs=int(os.environ.get("K_OT_BUFS", "2"))) as otpool,
            tc.tile_pool(name="mmps", bufs=int(os.environ.get("K_MM_BUFS", "2")), space="PSUM") as mmpool,
            tc.tile_pool(name="trps", bufs=int(os.environ.get("K_TR_BUFS", "2")), space="PSUM") as trpool,
        ):
            wmain_sb = constp.tile([128, 3, 96], BF16)
            nc.sync.dma_start(out=wmain_sb, in_=wmain_d)
            wleft_sb = constp.tile([49, 96], BF16)
            nc.sync.dma_start(out=wleft_sb, in_=wleft_d)
            identf = constp.tile([128, 128], F32)
            make_identity(nc, identf)
            ident = constp.tile([128, 128], BF16)
            nc.vector.tensor_copy(ident, identf)
            zscr16 = constp.tile([128, 16], F32)
            nc.vector.memset(zscr16, 0.0)

            nlrep = int(os.environ.get("K_LREP_BUFS", "2"))
            # persistent double-buffered lrep; ones row loaded once
            lreps = []
            for i in range(nlrep):
                lr = lpool.tile([49, T, RPITCH], BF16, name=f"lrep{i}")
                nc.sync.dma_start(out=_flat2(lr[48:49, :, :]),
                                  in_=ones_d[:, :T * RPITCH])
                lreps.append(lr)

            _lim = int(os.environ.get("K_CHUNK_LIMIT", "0"))
            _pairs = [(rep, b, c) for rep in range(reps)
                      for b in range(B) for c in range(NCHUNK)]
            if _lim:
                _pairs = _pairs[:_lim]
            for pi, (rep, b, chunk) in enumerate(_pairs):
                h0 = chunk * HC
                gp = [gppool.tile([128, 6, 16, SLOTS], F32, tag=f"gp{r}",
                                  name=f"gp{r}_{rep}_{b}_{chunk}")
                      for r in range(HC)]
                ot = otpool.tile([128, HC, COUT, T], F32, tag="ot")

                # one DMA per tap: [33, 2h, 130w] contiguous rows; dst
                # offset 1-dw bakes in the w-shift, src row range the
                # h-shift.
                mrep = mpool.tile([128, TP, RPITCH], BF16, tag="mrep")
                _split = int(os.environ.get("K_DMA_SPLIT", "0"))
                for g, (dh, dw) in enumerate(MAIN_TAPS):
                    dma_eng = nc.scalar if (_split and g % 2) else nc.sync
                    dma_eng.dma_start(
                        out=mrep[g * 16:(g + 1) * 16, :,
                                 1 - dw:1 - dw + 2 * WP],
                        in_=xs[b, :, :,
                               h0 + 1 + dh:h0 + 3 + dh, :].rearrange(
                            "c t h w -> c t (h w)"))
                lrep = lreps[pi % nlrep]
                for dtg in range(3):
                    nc.sync.dma_start(
                        out=lrep[dtg * 16:(dtg + 1) * 16, :, 0:2 * WP],
                        in_=xs[b, :, dtg:T + dtg,
                               h0 + 2:h0 + 4, :].rearrange(
                            "c t h w -> c t (h w)"))

                gc = gcpool.tile([96, T, NPIX], BF16, tag="gc")

                # conv matmuls + eviction, 4-t psum groups
                for gi, tg in enumerate(range(0, T, 4)):
                    gs = min(4, T - tg)
                    ps = mmpool.tile([96, 4, NPIX], F32, tag="mmps")
                    for tt in range(tg, tg + gs):
                        pslot = ps[:, tt - tg, :]
                        for dt in range(3):
                            nc.tensor.matmul(
                                pslot,
                                wmain_sb[:, dt, :],
                                _rhs_view(mrep, tt + dt),
                                start=(dt == 0), stop=False)
                        nc.tensor.matmul(
                            pslot,
                            wleft_sb,
                            _rhs_view(lrep, tt),
                            start=False, stop=True)
                    dst = gc[:, tg:tg + gs, :]
                    src = ps[:, :gs, :]
                    if ev_mode == 1 or (ev_mode == 0 and gi % 2 == 0):
                        nc.scalar.activation(dst, src, ACTF.Copy)
                    else:
                        nc.vector.tensor_copy(dst, src)

                # transpose to pixel-major + fused activation scatter
                for cg in range(0, T, csz):
                    cs = min(csz, T - cg)
                    for r in range(HC):
                        trp = trpool.tile([128, csz, 128], BF16, tag="trps")
                        for j in range(cs):
                            nc.tensor.transpose(
                                trp[:, j, 0:96],
                                gc[:, cg + j, r * W:(r + 1) * W],
                                ident[:96, :96])
                        tv = trp[:, :cs, :]
                        gpr = gp[r]
                        # tanh planes wx,X (cols 0:32), slot=t
                        nc.scalar.activation(
                            gpr[:, 0:2, :, cg:cg + cs],
                            tv[:, :, 0:32].rearrange(
                                "p t (g c) -> p g c t", g=2),
                            ACTF.Tanh)
                        # sigmoid fwd planes ft,rt (cols 32:64), slot=t
                        nc.scalar.activation(
                            gpr[:, 2:4, :, cg:cg + cs],
                            tv[:, :, 32:64].rearrange(
                                "p t (g c) -> p g c t", g=2),
                            ACTF.Sigmoid)
                        # sigmoid bwd planes ft2,rt2 (cols 64:96), slot=30-t
                        nc.scalar.activation(
                            _rev_last(gpr[:, PL_FT2:PL_RT2 + 1],
                                      count=cs, start=30 - cg),
                            tv[:, :, 64:96].rearrange(
                                "p t (g c) -> p g c t", g=2),
                            ACTF.Sigmoid)

                # scan phase for this (b, chunk)
                for r in range(HC):
                    gpr = gp[r]
                    nc.gpsimd.tensor_copy(gpr[:, PL_FT, :, 31:32],
                                          zscr16.unsqueeze(2))
                    nc.gpsimd.tensor_copy(gpr[:, PL_FT2, :, 31:32],
                                          zscr16.unsqueeze(2))

                    cf = scrp.tile([128, 16, SLOTS], F32, tag="cf")
                    cb = scrp.tile([128, 16, SLOTS], F32, tag="cb")
                    bb_ = scrp.tile([128, 16, SLOTS], F32, tag="bb")
                    wx_pl = gpr[:, PL_WX]
                    for (pf, wx_in, cdst) in (
                            (PL_FT, wx_pl[:, :, 1:31], cf),
                            (PL_FT2, _rev_last(wx_pl, 30, 29), cb)):
                        f_pl = gpr[:, pf]
                        nc.vector.tensor_scalar_sub(
                            bb_[:, :, 0:1], f_pl[:, :, 0:1], 1.0)
                        nc.vector.scalar_tensor_tensor(
                            out=bb_[:, :, 1:31], in0=f_pl[:, :, 1:31],
                            scalar=1.0, in1=wx_in,
                            op0=ALU.subtract, op1=ALU.mult)
                        nc.gpsimd.tensor_copy(bb_[:, :, 31:32],
                                              zscr16.unsqueeze(2))
                        nc.vector.tensor_tensor_scan(
                            out=_flat2(cdst), data0=_flat2(f_pl),
                            data1=_flat2(bb_), initial=0.0,
                            op0=ALU.mult, op1=ALU.subtract)

                    s1 = scrp.tile([128, 16, 31], F32, tag="s1")
                    s2 = scrp.tile([128, 16, SLOTS], F32, tag="s2")
                    dd = scrp.tile([128, 16, 31], F32, tag="dd")
                    ee = scrp.tile([128, 16, 31], F32, tag="ee")
                    nc.gpsimd.tensor_mul(
                        s1, gpr[:, PL_RT, :, 0:31], cf[:, :, 0:31])
                    nc.vector.tensor_mul(
                        s2[:, :, 0:31], gpr[:, PL_RT2, :, 0:31],
                        cb[:, :, 0:31])
                    nc.vector.tensor_add(
                        dd, gpr[:, PL_RT, :, 0:31],
                        _rev_last(gpr[:, PL_RT2], 31, 30))
                    nc.vector.scalar_tensor_tensor(
                        out=ee, in0=dd, scalar=2.0,
                        in1=gpr[:, PL_X, :, 0:31],
                        op0=ALU.subtract, op1=ALU.mult)
                    nc.gpsimd.tensor_add(s1, s1, _rev_last(s2, 31, 30))
                    nc.gpsimd.tensor_sub(ot[:, r], s1, ee)

                # output: [128w, c, t] per h-row -> contiguous DRAM run
                for r in range(HC):
                    nc.scalar.dma_start(
                        out=out_d[b, h0 + r, :, :, :],
                        in_=ot[:, r])
    nc.compile()
    return nc


_NC_CACHE = {}


def _get_nc(reps=1):
    if reps not in _NC_CACHE:
        _NC_CACHE[reps] = build_nc(reps)
    return _NC_CACHE[reps]


def make_host_inputs(x, conv_w, conv_b):
    """Pad x, permute/flatten weights, cast to bf16.
    Returns (xp, wmain, wleft, ones)."""
    import ml_dtypes
    bf16 = ml_dtypes.bfloat16

    x = np.asarray(x, np.float32)
    conv_w = np.asarray(conv_w, np.float32)
    conv_b = np.asarray(conv_b, np.float32)

    xp = np.zeros((B, CIN, TP, H + 2, WP), np.float32)
    xp[:, :, 1:1 + T, 1:1 + H, 1:1 + W] = x
    xp = xp.astype(bf16)

    wp = conv_w.reshape(6, COUT, CIN, 3, 3, 3)[GPERM].reshape(
        96, CIN, 3, 3, 3)
    bp = conv_b.reshape(6, COUT)[GPERM].reshape(96)

    wmain = np.zeros((128, 3, 96), np.float32)
    for g, (dh, dw) in enumerate(MAIN_TAPS):
        for dt in range(3):
            wmain[g * 16:(g + 1) * 16, dt, :] = wp[:, :, dt, dh + 1, dw + 1].T
    wleft = np.zeros((49, 96), np.float32)
    for dtg in range(3):
        wleft[dtg * 16:(dtg + 1) * 16, :] = wp[:, :, dtg, 2, 2].T
    wleft[48, :] = bp
    ones = np.ones((1, T * RPITCH), np.float32)
    return (xp, wmain.astype(bf16), wleft.astype(bf16), ones.astype(bf16))


def core_inputs(xp, wmain, wleft, ones, k):
    return {
        "xs": np.ascontiguousarray(
            xp[:, :, :, k * HSLAB:k * HSLAB + HSLAB + 2, :]),
        "wmain": wmain,
        "wleft": wleft,
        "ones": ones,
    }


def kernel(x, conv_w, conv_b):
    nc = _get_nc()
    xp, wmain, wleft, ones = make_host_inputs(x, conv_w, conv_b)
    in_maps = [core_inputs(xp, wmain, wleft, ones, k) for k in range(NCORES)]
    res = run_bass_kernel_spmd(nc, in_maps, list(range(NCORES)))
    outs = [res.results[k]["out"].transpose(0, 3, 4, 1, 2)
            for k in range(NCORES)]
    return np.concatenate(outs, axis=3)


# revision 11
# speedup vs baseline: 4.7000x; 1.1110x over previous
"""DoSRUconv Trainium2 kernel: 3x3x3 conv -> 6 gates -> bidirectional SRU scan.

Sharding: H (dim 3) split across 8 cores, 16 rows each; 3x3x3 conv halo is
resolved host-side by handing each core a zero-padded 18-row slab, so cores
are fully independent (no collectives).

Per-core pipeline (v2, bf16 datapath):
  - x and conv weights cast to bf16 on host; conv as bf16 matmuls over
    contraction (ci, dt, dh, dw) = 432(+bias row): 3 K=128 blocks (8
    (dh,dw)-taps x 16ci shifted replicas of x, dt via t-slice offset) + one
    K=49 leftover block ((1,1) tap x 3 dt + ones row carrying the bias);
    f32 PSUM accumulate.  bf16 => 1 cycle/row on PE at any N.
  - replica tiles hold [2h x 130w] contiguous row-pairs per t-slot (full-T,
    33 slots) at pitch 264; the per-tap (dh, dw) shift is baked in via the
    DMA's src row range and dst offset (1-dw), so one DMA per tap loads the
    whole band.  bf16 halves all HBM traffic.
  - PSUM evicted to bf16 gate tiles (DVE/ACT split), PE-transposed
    [96, 128] -> [128(w), 96] per (t, h-row).
  - tanh/sigmoid fused into the post-transpose plane scatter on the ACT
    engine (3 ops per (h-row, t-group): tanh cols 0:32, sigmoid 32:64,
    sigmoid 64:96 written t-reversed for the backward direction), so no
    separate activation pass exists.
  - SRU recurrence via tensor_tensor_scan in f32 (state = f*state - b with
    b=(f-1)*wx); 16 channel segments chained in one [128, 512] scan with
    zero-f/zero-b separator columns; backward direction stored t-reversed,
    reading wx through negative-stride views.
  - output written to DRAM as [B, H, W, C, T] f32 (one DMA per (b, chunk,
    h)); host transposes back to [B, C, T, H, W].
"""

import os

import numpy as np

import concourse.bass as bass
import concourse.mybir as mybir
import concourse.tile as tile
from concourse import bacc
from concourse.bass_utils import run_bass_kernel_spmd
from concourse.masks import make_identity

F32 = mybir.dt.float32
BF16 = mybir.dt.bfloat16
ALU = mybir.AluOpType
ACTF = mybir.ActivationFunctionType

B, CIN, COUT, T, H, W = 2, 16, 16, 31, 128, 128
NCORES = 8
HSLAB = H // NCORES                  # 16
HC = 2                               # h-rows per conv chunk (matmul N = HC*128)
NCHUNK = HSLAB // HC                 # 8
TP, WP = T + 2, W + 2                # padded dims: 33 t-slices, 130 w
SLOTS = 32                           # t-slots per channel segment (31 t + pad)
NPIX = HC * W                        # 256
RPITCH = 264                         # replica slot pitch: 2*130 + 2 slack

# 8 main taps (dh, dw), lexicographic minus (1,1); leftover tap is (1,1)
MAIN_TAPS = [(dh, dw) for dh in (-1, 0, 1) for dw in (-1, 0, 1)][:8]
# gate order [Wx, X, ft, rt, ft2, rt2] from reference split order
# [Wx, ft, ft2, rt, rt2, X]
GPERM = [0, 5, 1, 3, 2, 4]
# Gp plane indices (pixel-major gate store); tanh planes 0:2, sigmoid 2:6
PL_WX, PL_X, PL_FT, PL_RT, PL_FT2, PL_RT2 = range(6)


def _flat2(ap):
    return ap.rearrange("p a b -> p (a b)")


def _rev_last(ap, count=31, start=30):
    """View with the last (unit-stride) dim replaced by a reversed run of
    `count` starting at index `start` (descending)."""
    new_ap = [list(d) for d in ap.ap]
    assert new_ap[-1][0] == 1
    new_ap[-1] = [-1, count]
    return bass.AP(tensor=ap.tensor, offset=ap.offset + start, ap=new_ap)


def _rhs_view(rep, slot):
    """Matmul rhs [P, HC, W] from a replica tile [P, slots, RPITCH]:
    element (r, w) at slot*RPITCH + 2 + 130*r + w."""
    base = rep[:, slot]          # [P, RPITCH]
    ap = [list(base.ap[0]), [130, HC], [1, W]]
    return bass.AP(tensor=base.tensor, offset=base.offset + 2, ap=ap)


def build_nc(reps=1):
    nc = bacc.Bacc("TRN2", target_bir_lowering=False, debug=False)

    xs = nc.dram_tensor("xs", [B, CIN, TP, HSLAB + 2, WP], BF16,
                        kind="ExternalInput").ap()
    wmain_d = nc.dram_tensor("wmain", [128, 3, 96], BF16,
                             kind="ExternalInput").ap()
    wleft_d = nc.dram_tensor("wleft", [49, 96], BF16,
                             kind="ExternalInput").ap()
    ones_d = nc.dram_tensor("ones", [1, T * RPITCH], BF16,
                            kind="ExternalInput").ap()
    # stored [b, h, w, c, t]; host transposes back to [b, c, t, h, w]
    out_d = nc.dram_tensor("out", [B, HSLAB, W, COUT, T], F32,
                           kind="ExternalOutput").ap()

    ev_mode = int(os.environ.get("K_EVICT", "1"))   # 0 alt, 1 ACT, 2 DVE
    csz = int(os.environ.get("K_CS", "16"))

    with tile.TileContext(nc) as tc:
        with (
            tc.tile_pool(name="const", bufs=1) as constp,
            tc.tile_pool(name="mrep", bufs=int(os.environ.get("K_MREP_BUFS", "2"))) as mpool,
            tc.tile_pool(name="lrep", bufs=1) as lpool,
            tc.tile_pool(name="gc", bufs=int(os.environ.get("K_GC_BUFS", "2"))) as gcpool,
            tc.tile_pool(name="gp", bufs=int(os.environ.get("K_GP_BUFS", "2"))) as gppool,
            tc.tile_pool(name="scr", bufs=int(os.environ.get("K_SCR_BUFS", "2"))) as scrp,
            tc.tile_pool(name="ot", bufs=int(os.environ.get("K_OT_BUFS", "2"))) as otpool,
            tc.tile_pool(name="mmps", bufs=int(os.environ.get("K_MM_BUFS", "3")), space="PSUM") as mmpool,
            tc.tile_pool(name="trps", bufs=int(os.environ.get("K_TR_BUFS", "1")), space="PSUM") as trpool,
        ):
            wmain_sb = constp.tile([128, 3, 96], BF16)
            nc.sync.dma_start(out=wmain_sb, in_=wmain_d)
            wleft_sb = constp.tile([49, 96], BF16)
            nc.sync.dma_start(out=wleft_sb, in_=wleft_d)
            identf = constp.tile([128, 128], F32)
            make_identity(nc, identf)
            ident = constp.tile([128, 128], BF16)
            nc.vector.tensor_copy(ident, identf)
            zscr16 = constp.tile([128, 16], F32)
            nc.vector.memset(zscr16, 0.0)

            nlrep = int(os.environ.get("K_LREP_BUFS", "2"))
            # persistent double-buffered lrep; ones row loaded once
            lreps = []
            for i in range(nlrep):
                lr = lpool.tile([49, T, RPITCH], BF16, name=f"lrep{i}")
                nc.sync.dma_start(out=_flat2(lr[48:49, :, :]),
                                  in_=ones_d[:, :T * RPITCH])
                lreps.append(lr)

            _lim = int(os.environ.get("K_CHUNK_LIMIT", "0"))
            _pairs = [(rep, b, c) for rep in range(reps)
                      for b in range(B) for c in range(NCHUNK)]
            if _lim:
                _pairs = _pairs[:_lim]
            for pi, (rep, b, chunk) in enumerate(_pairs):
                h0 = chunk * HC
                gp = [gppool.tile([128, 6, 16, SLOTS], F32, tag=f"gp{r}",
                                  name=f"gp{r}_{rep}_{b}_{chunk}")
                      for r in range(HC)]
                ot = otpool.tile([128, HC, COUT, T], F32, tag="ot")

                # one DMA per tap: [33, 2h, 130w] contiguous rows; dst
                # offset 1-dw bakes in the w-shift, src row range the
                # h-shift.
                mrep = mpool.tile([128, TP, RPITCH], BF16, tag="mrep")
                _split = int(os.environ.get("K_DMA_SPLIT", "1"))
                for g, (dh, dw) in enumerate(MAIN_TAPS):
                    dma_eng = nc.scalar if (_split and g % 2) else nc.sync
                    dma_eng.dma_start(
                        out=mrep[g * 16:(g + 1) * 16, :,
                                 1 - dw:1 - dw + 2 * WP],
                        in_=xs[b, :, :,
                               h0 + 1 + dh:h0 + 3 + dh, :].rearrange(
                            "c t h w -> c t (h w)"))
                lrep = lreps[pi % nlrep]
                for dtg in range(3):
                    nc.sync.dma_start(
                        out=lrep[dtg * 16:(dtg + 1) * 16, :, 0:2 * WP],
                        in_=xs[b, :, dtg:T + dtg,
                               h0 + 2:h0 + 4, :].rearrange(
                            "c t h w -> c t (h w)"))

                gc = gcpool.tile([96, T, NPIX], BF16, tag="gc")

                # conv matmuls + eviction, 4-t psum groups
                for gi, tg in enumerate(range(0, T, 4)):
                    gs = min(4, T - tg)
                    ps = mmpool.tile([96, 4, NPIX], F32, tag="mmps")
                    for tt in range(tg, tg + gs):
                        pslot = ps[:, tt - tg, :]
                        for dt in range(3):
                            nc.tensor.matmul(
                                pslot,
                                wmain_sb[:, dt, :],
                                _rhs_view(mrep, tt + dt),
                                start=(dt == 0), stop=False)
                        nc.tensor.matmul(
                            pslot,
                            wleft_sb,
                            _rhs_view(lrep, tt),
                            start=False, stop=True)
                    dst = gc[:, tg:tg + gs, :]
                    src = ps[:, :gs, :]
                    if ev_mode == 1 or (ev_mode == 0 and gi % 2 == 0):
                        nc.scalar.activation(dst, src, ACTF.Copy)
                    else:
                        nc.vector.tensor_copy(dst, src)

                # transpose to pixel-major + fused activation scatter
                for cg in range(0, T, csz):
                    cs = min(csz, T - cg)
                    for r in range(HC):
                        trp = trpool.tile([128, csz, 128], BF16, tag="trps")
                        for j in range(cs):
                            nc.tensor.transpose(
                                trp[:, j, 0:96],
                                gc[:, cg + j, r * W:(r + 1) * W],
                                ident[:96, :96])
                        tv = trp[:, :cs, :]
                        gpr = gp[r]
                        # tanh planes wx,X (cols 0:32), slot=t
                        nc.scalar.activation(
                            gpr[:, 0:2, :, cg:cg + cs],
                            tv[:, :, 0:32].rearrange(
                                "p t (g c) -> p g c t", g=2),
                            ACTF.Tanh)
                        # sigmoid fwd planes ft,rt (cols 32:64), slot=t
                        nc.scalar.activation(
                            gpr[:, 2:4, :, cg:cg + cs],
                            tv[:, :, 32:64].rearrange(
                                "p t (g c) -> p g c t", g=2),
                            ACTF.Sigmoid)
                        # sigmoid bwd planes ft2,rt2 (cols 64:96), slot=30-t
                        nc.scalar.activation(
                            _rev_last(gpr[:, PL_FT2:PL_RT2 + 1],
                                      count=cs, start=30 - cg),
                            tv[:, :, 64:96].rearrange(
                                "p t (g c) -> p g c t", g=2),
                            ACTF.Sigmoid)

                # scan phase for this (b, chunk)
                for r in range(HC):
                    gpr = gp[r]
                    nc.gpsimd.tensor_copy(gpr[:, PL_FT, :, 31:32],
                                          zscr16.unsqueeze(2))
                    nc.gpsimd.tensor_copy(gpr[:, PL_FT2, :, 31:32],
                                          zscr16.unsqueeze(2))

                    cf = scrp.tile([128, 16, SLOTS], F32, tag="cf")
                    cb = scrp.tile([128, 16, SLOTS], F32, tag="cb")
                    bb_ = scrp.tile([128, 16, SLOTS], F32, tag="bb")
                    wx_pl = gpr[:, PL_WX]
                    for (pf, wx_in, cdst) in (
                            (PL_FT, wx_pl[:, :, 1:31], cf),
                            (PL_FT2, _rev_last(wx_pl, 30, 29), cb)):
                        f_pl = gpr[:, pf]
                        nc.vector.tensor_scalar_sub(
                            bb_[:, :, 0:1], f_pl[:, :, 0:1], 1.0)
                        nc.vector.scalar_tensor_tensor(
                            out=bb_[:, :, 1:31], in0=f_pl[:, :, 1:31],
                            scalar=1.0, in1=wx_in,
                            op0=ALU.subtract, op1=ALU.mult)
                        nc.gpsimd.tensor_copy(bb_[:, :, 31:32],
                                              zscr16.unsqueeze(2))
                        nc.vector.tensor_tensor_scan(
                            out=_flat2(cdst), data0=_flat2(f_pl),
                            data1=_flat2(bb_), initial=0.0,
                            op0=ALU.mult, op1=ALU.subtract)

                    s1 = scrp.tile([128, 16, 31], F32, tag="s1")
                    s2 = scrp.tile([128, 16, SLOTS], F32, tag="s2")
                    dd = scrp.tile([128, 16, 31], F32, tag="dd")
                    ee = scrp.tile([128, 16, 31], F32, tag="ee")
                    nc.gpsimd.tensor_mul(
                        s1, gpr[:, PL_RT, :, 0:31], cf[:, :, 0:31])
                    nc.vector.tensor_mul(
                        s2[:, :, 0:31], gpr[:, PL_RT2, :, 0:31],
                        cb[:, :, 0:31])
                    nc.vector.tensor_add(
                        dd, gpr[:, PL_RT, :, 0:31],
                        _rev_last(gpr[:, PL_RT2], 31, 30))
                    nc.vector.scalar_tensor_tensor(
                        out=ee, in0=dd, scalar=2.0,
                        in1=gpr[:, PL_X, :, 0:31],
                        op0=ALU.subtract, op1=ALU.mult)
                    nc.gpsimd.tensor_add(s1, s1, _rev_last(s2, 31, 30))
                    nc.gpsimd.tensor_sub(ot[:, r], s1, ee)

                # output: [128w, c, t] per h-row -> contiguous DRAM run
                for r in range(HC):
                    nc.scalar.dma_start(
                        out=out_d[b, h0 + r, :, :, :],
                        in_=ot[:, r])
    nc.compile()
    return nc


_NC_CACHE = {}


def _get_nc(reps=1):
    if reps not in _NC_CACHE:
        _NC_CACHE[reps] = build_nc(reps)
    return _NC_CACHE[reps]


def make_host_inputs(x, conv_w, conv_b):
    """Pad x, permute/flatten weights, cast to bf16.
    Returns (xp, wmain, wleft, ones)."""
    import ml_dtypes
    bf16 = ml_dtypes.bfloat16

    x = np.asarray(x, np.float32)
    conv_w = np.asarray(conv_w, np.float32)
    conv_b = np.asarray(conv_b, np.float32)

    xp = np.zeros((B, CIN, TP, H + 2, WP), np.float32)
    xp[:, :, 1:1 + T, 1:1 + H, 1:1 + W] = x
    xp = xp.astype(bf16)

    wp = conv_w.reshape(6, COUT, CIN, 3, 3, 3)[GPERM].reshape(
        96, CIN, 3, 3, 3)
    bp = conv_b.reshape(6, COUT)[GPERM].reshape(96)

    wmain = np.zeros((128, 3, 96), np.float32)
    for g, (dh, dw) in enumerate(MAIN_TAPS):
        for dt in range(3):
            wmain[g * 16:(g + 1) * 16, dt, :] = wp[:, :, dt, dh + 1, dw + 1].T
    wleft = np.zeros((49, 96), np.float32)
    for dtg in range(3):
        wleft[dtg * 16:(dtg + 1) * 16, :] = wp[:, :, dtg, 2, 2].T
    wleft[48, :] = bp
    ones = np.ones((1, T * RPITCH), np.float32)
    return (xp, wmain.astype(bf16), wleft.astype(bf16), ones.astype(bf16))


def core_inputs(xp, wmain, wleft, ones, k):
    return {
        "xs": np.ascontiguousarray(
            xp[:, :, :, k * HSLAB:k * HSLAB + HSLAB + 2, :]),
        "wmain": wmain,
        "wleft": wleft,
        "ones": ones,
    }


def kernel(x, conv_w, conv_b):
    nc = _get_nc()
    xp, wmain, wleft, ones = make_host_inputs(x, conv_w, conv_b)
    in_maps = [core_inputs(xp, wmain, wleft, ones, k) for k in range(NCORES)]
    res = run_bass_kernel_spmd(nc, in_maps, list(range(NCORES)))
    outs = [res.results[k]["out"].transpose(0, 3, 4, 1, 2)
            for k in range(NCORES)]
    return np.concatenate(outs, axis=3)
